# revision 29
# baseline (speedup 1.0000x reference)
"""AutoCorrelation kernel for Trainium2, 8 NeuronCores.

Math per (b, h) pair with X = x[b, :, h*64:(h+1)*64]  [T=2048, hd=64]:
  Xc = X - mean_T(X)
  S  = Xc @ Xc.T                  (symmetric!)
  P  = softmax(S, axis=-1)
  out = P @ X

Implementation exploits symmetry of E = exp(S - 64):
  out[t] = (sum_T' E[t,T'] X[T']) / (sum_T' E[t,T'])
and E == E.T, so the row-blocks of E computed with t on partitions can be
used directly as the *streaming* operand of the PV matmul (lhsT = [X | 1]),
which also yields the softmax denominator L in output row 64. No transposes
of the attention matrix are ever needed.

Engine plan per core (8 independent (b,h) pairs, data parallel across 8
cores): S-matmuls run 2x via PE row-tiling (K=64 on tiles T0/T8); exp is
split between ScalarE (table exp) and VectorE (Schraudolph bf16 bit-trick
with saturating f32->u16 convert); the PV matmuls for the previous pair are
spread between S blocks so ScalarE never starves; prep (DMA, paired
transposes, centering) for the next pair is interleaved too.
"""

import numpy as np

NCORES = 8
B, T, D, H = 4, 2048, 1024, 16
HD = D // H            # 64
PAIRS = B * H          # 64
PPC = PAIRS // NCORES  # 8 pairs per core
KT = T // 128          # 16 row-blocks of 128

# exp blocks computed on VectorE via bf16 Schraudolph bit-trick (rest: ScalarE)
DVE_BLOCKS = frozenset({2, 4, 7, 9, 12, 14})
SCHRAUD_A = 128.0 / float(np.log(2.0))               # 184.6649...
SCHRAUD_B = 127.0 * 128.0 - 5.25 - 64.0 * SCHRAUD_A  # bf16 bits bias, folds exp(-64)

_CACHE = {}


def _build_nc():
    import concourse.bass as bass  # noqa: F401
    import concourse.tile as tile
    from concourse import bacc, mybir
    from concourse.masks import make_identity

    f32 = mybir.dt.float32
    bf16 = mybir.dt.bfloat16
    u16 = mybir.dt.uint16
    ADD = mybir.AluOpType.add
    MULT = mybir.AluOpType.mult
    EXP = mybir.ActivationFunctionType.Exp

    nc = bacc.Bacc(None)
    x_ext = nc.declare_dram_parameter("x", [PPC, T, HD], f32, isOutput=False)
    o_ext = nc.declare_dram_parameter("out", [PPC, T, HD], f32, isOutput=True)

    x_t = x_ext.ap().rearrange("p (ko pp) d -> p pp ko d", pp=128)
    o_t = o_ext.ap().rearrange("p (ko pp) d -> p pp ko d", pp=128)

    with tile.TileContext(nc) as tc:
        with (
            tc.tile_pool(name="const", bufs=1) as constp,
            tc.tile_pool(name="xst", bufs=2) as xstp,
            tc.tile_pool(name="stage", bufs=2) as stagep,
            tc.tile_pool(name="xct", bufs=2) as xctp,
            tc.tile_pool(name="vb", bufs=2) as vbp,
            tc.tile_pool(name="eb", bufs=2) as ebp,
            tc.tile_pool(name="osb", bufs=2) as osbp,
            tc.tile_pool(name="sbt", bufs=2) as sbtp,
            tc.tile_pool(name="small", bufs=4) as smallp,
            tc.tile_pool(name="psS", bufs=3, space="PSUM") as psSp,
            tc.tile_pool(name="psM", bufs=2, space="PSUM") as psMp,
        ):
            ident = constp.tile([128, 128], f32)
            make_identity(nc, ident)
            ones = constp.tile([128, 1], f32)
            nc.vector.memset(ones, 1.0)
            neg64 = constp.tile([128, 1], f32)
            nc.vector.memset(neg64, -64.0)

            state = {}
            xst_tiles = {}

            def emit_dma_in(p):
                xst = xstp.tile([128, KT, HD], f32, tag="xst")
                nc.gpsimd.dma_start(xst, x_t[p])
                xst_tiles[p] = xst

            def emit_prep_a(p):
                # V (with ones column) + packed bf16 copy for DMA-transposes
                xst = xst_tiles[p]
                vb = vbp.tile([128, KT, HD + 1], bf16, tag="vb")
                nc.vector.memset(vb[:, :, HD : HD + 1], 1.0)
                nc.vector.tensor_copy(vb[:, :, 0:HD], xst)
                xb = stagep.tile([128, KT * HD], bf16, tag="xb")
                nc.vector.tensor_copy(
                    xb.rearrange("p (k d) -> p k d", d=HD), xst
                )
                stage = stagep.tile([128, 2 * 512], bf16, tag="stage")
                xct = xctp.tile([128, T], bf16, tag="xct")
                E = ebp.tile([128, KT, T], bf16, tag="eb")
                osb = osbp.tile([128, KT, HD], f32, tag="osb")
                osb_bf = osbp.tile([128, KT, 80], bf16, tag="osb_bf")
                state[p] = {
                    "E": E, "vb": vb, "osb": osb, "xct": xct,
                    "stage": stage, "xb": xb, "osb_bf": osb_bf,
                }
                xst_tiles.pop(p)

            def emit_prep_b(p):
                # XT via one blocked DMA transpose: out[p, b, f] = in[f, b*128+p]
                # (partitions 0:64 get even k-tile's d, 64:128 odd k-tile's d)
                xb = state[p]["xb"]
                stage = state[p]["stage"]
                nc.sync.dma_start_transpose(
                    stage.rearrange("p (q f) -> p q f", f=128), xb
                )

            def emit_prep_b2(p):
                # mean over T from the transposed stage (free-axis reduce),
                # then center the stage in place
                stage = state[p]["stage"]
                part = smallp.tile([128, 1], f32, tag="part")
                nc.vector.tensor_reduce(
                    part, stage.rearrange("p (q f) -> p q f", f=128),
                    mybir.AxisListType.XY, ADD,
                )
                ptop = smallp.tile([HD, 1], f32, tag="ptop")
                nc.gpsimd.dma_start(ptop, part[HD:128])
                mufull = smallp.tile([128, 1], f32, tag="mufull")
                nc.vector.tensor_tensor(part[0:HD], part[0:HD], ptop, ADD)
                nc.scalar.mul(mufull[0:HD], part[0:HD], -1.0 / T)
                nc.gpsimd.dma_start(mufull[HD:128], mufull[0:HD])
                nc.vector.tensor_scalar(stage, stage, mufull, None, ADD)
                state[p]["mufull"] = mufull

            def emit_prep_c(p):
                # shuffle stage -> xct (both halves get all 16 k-tiles)
                stage = state[p]["stage"]
                xct = state[p]["xct"]
                sg = stage.rearrange("p (q f) -> p q f", f=128)
                xg = xct.rearrange("p (k f) -> p k f", f=128)
                nc.gpsimd.dma_start(xg[0:HD, 0:KT:2, :], sg[0:HD])
                nc.gpsimd.dma_start(xg[0:HD, 1:KT:2, :], sg[HD:128])
                nc.gpsimd.dma_start(xg[HD:128, 0:KT:2, :], sg[0:HD])
                nc.gpsimd.dma_start(xg[HD:128, 1:KT:2, :], sg[HD:128])

            def emit_s_exp(p, m):
                E = state[p]["E"]
                xct = state[p]["xct"]
                ms = slice(m * 128, (m + 1) * 128)
                # two half-blocks: h0 = cols 0:1024 on PE rows 0:63 (T0),
                # h1 = cols 1024:2048 on rows 64:127 (T8); T0/T8 concurrent.
                psh = [
                    psSp.tile([128, T // 2], f32, tag="psS", name=f"psS{h}")
                    for h in range(2)
                ]
                for n in range(2):
                    nc.tensor.matmul(
                        psh[0][:, n * 512 : (n + 1) * 512],
                        lhsT=xct[0:HD, ms],
                        rhs=xct[0:HD, n * 512 : (n + 1) * 512],
                        start=True, stop=True, tile_position=(0, 0),
                    )
                    nc.tensor.matmul(
                        psh[1][:, n * 512 : (n + 1) * 512],
                        lhsT=xct[HD:128, ms],
                        rhs=xct[HD:128, 1024 + n * 512 : 1024 + (n + 1) * 512],
                        start=True, stop=True, tile_position=(64, 0),
                    )
                for h in range(2):
                    eview = E[:, m, h * 1024 : (h + 1) * 1024]
                    if m in DVE_BLOCKS:
                        # Schraudolph in bf16 bit-space; f32->u16 convert
                        # saturates negatives to 0 (== exp underflow).
                        nc.vector.tensor_scalar(
                            eview.bitcast(u16), psh[h], SCHRAUD_A, SCHRAUD_B,
                            MULT, ADD,
                        )
                    else:
                        nc.scalar.activation(
                            eview, psh[h], EXP, bias=neg64, scale=1.0
                        )

            pv_live = {}

            def emit_pv_part(q, c, part):
                # 8 of the 16 accumulating PV matmuls for chunk c of pair q
                E, vb = state[q]["E"], state[q]["vb"]
                cs = slice(c * 512, (c + 1) * 512)
                if part == 0:
                    pv_live["ps"] = psMp.tile(
                        [HD + 1, 512], f32, tag="mix", name="pspv"
                    )
                pspv = pv_live["ps"]
                for kk in range(8):
                    k = part * 8 + kk
                    nc.tensor.matmul(
                        pspv,
                        lhsT=vb[:, k, :],
                        rhs=E[:, k, cs],
                        start=(k == 0), stop=(k == KT - 1),
                        skip_group_check=True,
                    )

            def emit_pv_tail(q, c):
                # evacuate PV psum as bf16, transpose back via DMA xbar,
                # then scale rows by 1/L (L rides in psum row 64)
                osb = state[q]["osb"]
                osb_bf = state[q]["osb_bf"]
                pspv = pv_live.pop("ps")
                sbt = sbtp.tile([80, 512], bf16, tag="sbt")
                nc.scalar.copy(sbt[0 : HD + 1], pspv)
                # block transpose via DMA xbar; row 64 (L) rides along,
                # rows 65:80 are padding for the 16-row xbar granularity.
                # scalar HWDGE ring: issues right after ScalarE's sbt copy.
                nc.scalar.dma_start_transpose(
                    osb_bf[:, c * 4 : (c + 1) * 4, :], sbt
                )
                lrec = smallp.tile([128, 4], f32, tag="lrec")
                nc.vector.reciprocal(lrec, osb_bf[:, c * 4 : (c + 1) * 4, HD])
                nc.vector.tensor_tensor(
                    osb[:, c * 4 : (c + 1) * 4, :],
                    osb_bf[:, c * 4 : (c + 1) * 4, 0:HD],
                    lrec[:, :, None].to_broadcast([128, 4, HD]), MULT,
                )

            emit_dma_in(0)
            emit_prep_a(0)
            emit_prep_b(0)
            emit_prep_b2(0)
            emit_prep_c(0)
            for it in range(PPC + 1):
                for m in range(KT):
                    if it < PPC:
                        emit_s_exp(it, m)
                    if it > 0 and m % 2 == 1:
                        emit_pv_part(it - 1, m // 4, (m % 4) // 2)
                        if m % 4 == 3:
                            emit_pv_tail(it - 1, m // 4)
                    if it + 1 < PPC:
                        if m == 0:
                            emit_dma_in(it + 1)
                        elif m == 7:
                            emit_prep_a(it + 1)
                        elif m == 8:
                            emit_prep_b(it + 1)
                        elif m == 10:
                            emit_prep_b2(it + 1)
                        elif m == 11:
                            emit_prep_c(it + 1)
                if it > 0:
                    osb = state[it - 1]["osb"]
                    nc.gpsimd.dma_start(o_t[it - 1], osb)
                    state.pop(it - 1)
    nc.compile()
    return nc


def _get_nc():
    if "nc" not in _CACHE:
        _CACHE["nc"] = _build_nc()
    return _CACHE["nc"]


def kernel(x: np.ndarray) -> np.ndarray:
    from concourse.bass_utils import run_bass_kernel_spmd

    nc = _get_nc()
    x = np.asarray(x, dtype=np.float32)
    xh = (
        x.reshape(B, T, H, HD).transpose(0, 2, 1, 3).reshape(PAIRS, T, HD)
    )
    in_maps = [
        {"x": np.ascontiguousarray(xh[i * PPC : (i + 1) * PPC])}
        for i in range(NCORES)
    ]
    res = run_bass_kernel_spmd(nc, in_maps, core_ids=list(range(NCORES)))
    outs = np.concatenate(
        [np.asarray(res.results[i]["out"]) for i in range(NCORES)], axis=0
    )
    return (
        outs.reshape(B, H, T, HD).transpose(0, 2, 1, 3).reshape(B, T, D)
    ).astype(np.float32)
